# revision 1
# baseline (speedup 1.0000x reference)
"""Trainium2 Bass kernel for causal bilinear self-attention (diagonal variant).

Computes, per (b, head):
    scores[t, s] = h[b, t] @ A[head] @ h[b, s]        (causal: s <= t)
    attn = softmax(scores, axis=-1)
    out[b, head, t, :] = attn[t, t] * h[b, t, :]
returned reshaped row-major to (B, T, H*d)  (faithful torch .view semantics).

Only the diagonal of the attention matrix is needed, so the kernel computes
flash-style per-row max / sum-of-exp over the causal triangle plus the
diagonal score, never materializing attention in HBM.

Sharding: 16 (b, head) pairs across 8 cores -> core c handles b = c // 4,
heads 2*(c%4) and 2*(c%4)+1.  Each core receives h[b] (4 MB) and its two
A matrices; outputs its two (T, d) slices.

Precision: the TensorE fp32 path costs 4 cyc/row; float32r (TF32-like,
11-bit mantissa, exact MAC on rounded inputs) costs 1 cyc/row at N>=256.
Splitting an fp32 value x into xh = round_f32r(x), xl = x - xh (both exactly
representable in f32r) lets multi-pass f32r matmuls recover near-fp32
precision at 2-3 cyc/row.  STAGE1/STAGE2 below select the scheme per stage.

Hardware notes (found empirically on this axon/neuronxcc toolchain):
  - tensor_tensor_reduce with a PSUM input crashes the device; so does an
    ACT read of a PSUM region modified in place by the DVE.  PSUM is
    therefore written only by the PE and read only by DVE copy-class ops;
    all masking / softmax arithmetic happens on SBUF copies.
  - mask constants are DMA'd from host inputs (no gpsimd affine_select).
"""

import os
import sys

try:
    import concourse.bass  # noqa: F401
except ImportError:  # pragma: no cover
    sys.path.insert(0, "/opt/trn_rl_repo")

import numpy as np

import concourse.bass as bass  # noqa: F401
import concourse.tile as tile
from concourse import bacc, bass_utils, mybir

B, T, D, H = 2, 2048, 512, 8
NCORES = 8
P = 128
NT = T // P      # 16 row tiles
ND = D // P      # 4 contraction chunks
SCH = 512        # s-chunk width (one PSUM bank of fp32)
NEG = -1.0e30

f32 = mybir.dt.float32
f32r = mybir.dt.float32r

# stage1 (g = h @ A): "f32" exact 4 cyc/row | "r3" ~exact 3 | "r2" 2 | "r1" 1
# stage2 (scores = g @ h^T): "f32" 4 | "r3" ~exact 3 | "r2" 2 | "r1" 1
# Default r3/r3: 3-pass f32r split per stage -> ~fp32 accuracy (measured
# 2.3e-6 rel-to-max vs fp64 reference) at ~317 us/core predicted, vs 381 us
# for the plain fp32 path (0.0 measured error) and 154 us for r1/r1 (1.4e-3).
STAGE1 = os.environ.get("BK_STAGE1", "r3")
STAGE2 = os.environ.get("BK_STAGE2", "r3")


def build_nc(stage1=None, stage2=None):
    stage1 = stage1 or STAGE1
    stage2 = stage2 or STAGE2
    assert stage1 in ("f32", "r3", "r2", "r1") and stage2 in ("f32", "r3", "r2", "r1")
    s2_r = stage2 != "f32"
    need_hT32 = stage1 == "f32" or stage2 == "f32"
    need_hTr = stage1 != "f32" or stage2 != "f32"
    need_hTl = stage1 == "r3" or stage2 == "r3"
    need_Ar = stage1 != "f32"
    need_Al = stage1 in ("r3", "r2")

    nc = bacc.Bacc("TRN2", target_bir_lowering=False, debug=False)
    hb = nc.dram_tensor("hb", [T, D], f32, kind="ExternalInput")
    A2 = nc.dram_tensor("A2", [2, D, D], f32, kind="ExternalInput")
    cmaskd = nc.dram_tensor("cmaskd", [P, P], f32, kind="ExternalInput")
    identd = nc.dram_tensor("identd", [P, P], f32, kind="ExternalInput")
    out2 = nc.dram_tensor("out2", [2, T, D], f32, kind="ExternalOutput")
    hb_t = hb[:].rearrange("(n p) d -> p n d", p=P)  # [128, 16, 512] view

    with tile.TileContext(nc) as tc:
        with (
            tc.tile_pool(name="const", bufs=1) as constp,
            tc.tile_pool(name="big", bufs=1) as big,
            tc.tile_pool(name="gpool", bufs=1) as gpool,
            tc.tile_pool(name="hin", bufs=3) as hin,
            tc.tile_pool(name="psum", bufs=8, space="PSUM") as psum,
            tc.tile_pool(name="scs", bufs=3) as scs,
            tc.tile_pool(name="escr", bufs=2) as escr,
            tc.tile_pool(name="stats", bufs=4) as stats,
            tc.tile_pool(name="outp", bufs=2) as outp,
        ):
            ident = constp.tile([P, P], f32)
            nc.gpsimd.dma_start(out=ident, in_=identd[:])
            cmask = constp.tile([P, P], f32)
            nc.gpsimd.dma_start(out=cmask, in_=cmaskd[:])

            # A: A_sb[p, hd, c, e] = A[hd, c*128 + p, e]
            A_sb = big.tile([P, 2, ND, D], f32)
            nc.gpsimd.dma_start(
                out=A_sb, in_=A2[:].rearrange("h (c p) e -> p h c e", p=P)
            )
            if need_Ar:
                A_r = big.tile([P, 2, ND, D], f32r)
                nc.vector.tensor_copy(A_r, A_sb)
            if need_Al:
                A_l = big.tile([P, 2, ND, D], f32r)
                nc.vector.tensor_sub(A_l, A_sb, A_r.bitcast(f32))

            # h^T: hT*[p, c, t] = h[t, c*128 + p], via PE transpose
            NS = T // SCH
            def _mk(name, dt_):
                return [[big.tile([P, SCH], dt_, name=f"{name}_{c}_{s}")
                         for s in range(NS)] for c in range(ND)]
            hT32 = _mk("hT32", f32) if need_hT32 else None
            hTr = _mk("hTr", f32r) if need_hTr else None
            hTl = _mk("hTl", f32r) if need_hTl else None

            def _hT(tens, c, lo, width):
                # contiguous [lo, lo+width) slice lies within one 512 slice
                s, off = lo // SCH, lo % SCH
                return tens[c][s][:, off : off + width]
            for i in range(NT):
                hrow = hin.tile([P, D], f32, tag="hrow")
                nc.sync.dma_start(out=hrow, in_=hb_t[:, i, :])
                for c in range(ND):
                    pt = psum.tile([P, P], f32, tag="ps")
                    nc.tensor.transpose(pt, hrow[:, c * P : (c + 1) * P], ident)
                    if need_hT32:
                        nc.vector.tensor_copy(_hT(hT32, c, i * P, P), pt)
                    if need_hTr:
                        nc.vector.tensor_copy(_hT(hTr, c, i * P, P), pt)
                    if need_hTl:
                        nc.vector.tensor_sub(
                            _hT(hTl, c, i * P, P), pt,
                            _hT(hTr, c, i * P, P).bitcast(f32)
                        )

            for hd in range(2):
                # ---- stage 1: gT[e, t] = sum_d A[d, e] * hT[d, t] ----
                need_g32 = stage2 == "f32"
                need_gh = s2_r
                need_gl = stage2 in ("r3", "r2")
                gT32 = gpool.tile([P, ND, T], f32, tag="g32", name="gT32") if need_g32 else None
                gTh = gpool.tile([P, ND, T], f32r, tag="gh", name="gTh") if need_gh else None
                gTl = gpool.tile([P, ND, T], f32r, tag="gl", name="gTl") if need_gl else None

                if stage1 == "f32":
                    s1_passes = [(A_sb, hT32)]
                elif stage1 == "r1":
                    s1_passes = [(A_r, hTr)]
                elif stage1 == "r2":
                    s1_passes = [(A_r, hTr), (A_l, hTr)]
                else:  # r3
                    s1_passes = [(A_r, hTr), (A_l, hTr), (A_r, hTl)]

                for ec in range(ND):
                    ecs = slice(ec * P, (ec + 1) * P)
                    for tsl in range(T // SCH):
                        ts_ = slice(tsl * SCH, (tsl + 1) * SCH)
                        pg = psum.tile([P, SCH], f32, tag="ps")
                        nmm = len(s1_passes) * ND
                        k = 0
                        for lhs_src, rhs_src in s1_passes:
                            for dc in range(ND):
                                nc.tensor.matmul(
                                    pg,
                                    lhs_src[:, hd, dc, ecs],
                                    rhs_src[dc][tsl],
                                    start=(k == 0),
                                    stop=(k == nmm - 1),
                                )
                                k += 1
                        if gT32 is not None:
                            nc.vector.tensor_copy(gT32[:, ec, ts_], pg)
                        if gTh is not None:
                            nc.vector.tensor_copy(gTh[:, ec, ts_], pg)
                        if gTl is not None:
                            nc.vector.tensor_sub(
                                gTl[:, ec, ts_], pg, gTh[:, ec, ts_].bitcast(f32)
                            )

                if stage2 == "f32":
                    s2_passes = [(gT32, hT32)]
                elif stage2 == "r3":
                    s2_passes = [(gTh, hTr), (gTl, hTr), (gTh, hTl)]
                elif stage2 == "r2":
                    s2_passes = [(gTh, hTr), (gTl, hTr)]
                else:
                    s2_passes = [(gTh, hTr)]

                # ---- stage 2 + softmax diag, per row tile ----
                for i in range(NT):
                    nch = i // 4 + 1
                    its = slice(i * P, (i + 1) * P)
                    dcol = (i % 4) * P       # diag block start within last chunk
                    wlast = (i % 4 + 1) * P  # causal width of last chunk
                    # f32r matmuls need moving dim >= 256 for full rate; widen
                    # the 128-wide matmul (extra cols never copied out of PSUM)
                    w_mm = max(wlast, 2 * P) if s2_r else wlast

                    m4 = stats.tile([P, 4], f32, tag="m4")
                    lp = stats.tile([P, 4], f32, tag="lp")
                    chunks = []
                    for j in range(nch):
                        last = j == nch - 1
                        w = w_mm if last else SCH
                        wc = wlast if last else SCH  # causal (copied) width
                        ps = psum.tile([P, SCH], f32, tag="ps")
                        nmm = len(s2_passes) * ND
                        k = 0
                        for lhs_src, rhs_src in s2_passes:
                            for ec in range(ND):
                                nc.tensor.matmul(
                                    ps[:, :w],
                                    lhs_src[:, ec, its],
                                    rhs_src[ec][j][:, :w],
                                    start=(k == 0),
                                    stop=(k == nmm - 1),
                                )
                                k += 1
                        if last:
                            # diag chunk: SBUF copy + causal mask (PSUM must
                            # stay PE-written-only for ACT readers)
                            sc = scs.tile([P, SCH], f32, tag="sc")
                            nc.vector.tensor_copy(sc[:, :wc], ps[:, :wc])
                            nc.vector.tensor_add(
                                sc[:, dcol : dcol + P], sc[:, dcol : dcol + P], cmask
                            )
                            src_t = sc
                        else:
                            src_t = ps
                        nc.vector.reduce_max(
                            out=m4[:, j : j + 1], in_=src_t[:, :wc],
                            axis=mybir.AxisListType.X,
                        )
                        chunks.append((src_t, wc))

                    nm = stats.tile([P, 1], f32, tag="nm")
                    nc.vector.reduce_max(
                        out=nm, in_=m4[:, :nch], axis=mybir.AxisListType.X, negate=True
                    )
                    ex_last = None
                    for j, (sc, wc) in enumerate(chunks):
                        ex = escr.tile([P, SCH], f32, tag="ex")
                        nc.scalar.activation(
                            out=ex[:, :wc],
                            in_=sc[:, :wc],
                            func=mybir.ActivationFunctionType.Exp,
                            bias=nm,
                            scale=1.0,
                            accum_out=lp[:, j : j + 1],
                        )
                        if j == nch - 1:
                            ex_last = ex
                    # diag of exp block: mul by identity then row-reduce
                    dscr = stats.tile([P, P], f32, tag="dscr")
                    nc.vector.tensor_mul(dscr, ex_last[:, dcol : dcol + P], ident)
                    ediag = stats.tile([P, 1], f32, tag="ediag")
                    nc.vector.reduce_sum(
                        out=ediag, in_=dscr, axis=mybir.AxisListType.X
                    )
                    lsum = stats.tile([P, 1], f32, tag="lsum")
                    nc.vector.reduce_sum(
                        out=lsum, in_=lp[:, :nch], axis=mybir.AxisListType.X
                    )
                    rl = stats.tile([P, 1], f32, tag="rl")
                    nc.vector.reciprocal(rl, lsum)
                    datt = stats.tile([P, 1], f32, tag="datt")
                    nc.vector.tensor_mul(datt, ediag, rl)

                    hrow2 = hin.tile([P, D], f32, tag="hrow2")
                    nc.sync.dma_start(out=hrow2, in_=hb_t[:, i, :])
                    ot = outp.tile([P, D], f32, tag="ot")
                    nc.vector.tensor_scalar_mul(ot, hrow2, datt)
                    nc.sync.dma_start(out=out2[hd, its, :], in_=ot)

    nc.compile()
    return nc


_NC_CACHE = {}


def _get_nc(stage1=None, stage2=None):
    key = (stage1 or STAGE1, stage2 or STAGE2)
    if key not in _NC_CACHE:
        _NC_CACHE[key] = build_nc(*key)
    return _NC_CACHE[key]


def _consts():
    cmask = np.triu(np.full((P, P), NEG, np.float32), 1)
    ident = np.eye(P, dtype=np.float32)
    return cmask, ident


def make_in_maps(h, A):
    h = np.ascontiguousarray(h, dtype=np.float32)
    A = np.ascontiguousarray(A, dtype=np.float32)
    cmask, ident = _consts()
    in_maps = []
    for c in range(NCORES):
        b = c // 4
        h0 = 2 * (c % 4)
        in_maps.append({"hb": h[b], "A2": np.ascontiguousarray(A[h0 : h0 + 2]),
                        "cmaskd": cmask, "identd": ident})
    return in_maps


def assemble(results):
    full = np.empty((B, H, T, D), dtype=np.float32)
    for c in range(NCORES):
        b = c // 4
        h0 = 2 * (c % 4)
        o = results[c]["out2"]
        full[b, h0] = o[0]
        full[b, h0 + 1] = o[1]
    return full.reshape(B, T, H * D)


def kernel(h, A):
    nc = _get_nc()
    res = bass_utils.run_bass_kernel_spmd(
        nc, make_in_maps(h, A), core_ids=list(range(NCORES))
    )
    return assemble(res.results)



# revision 2
# speedup vs baseline: 1.6856x; 1.6856x over previous
"""Trainium2 Bass kernel for causal bilinear self-attention (diagonal variant).

Computes, per (b, head):
    scores[t, s] = h[b, t] @ A[head] @ h[b, s]        (causal: s <= t)
    attn = softmax(scores, axis=-1)
    out[b, head, t, :] = attn[t, t] * h[b, t, :]
returned reshaped row-major to (B, T, H*d)  (faithful torch .view semantics).

Only the diagonal of the attention matrix is needed:
    attn[t, t] = exp(s_tt - c_t) / sum_{s<=t} exp(s_ts - c_t)   for any shift c_t.
Choosing c_t = s_tt (the raw diagonal score) instead of the row max makes the
numerator exp(0) = 1, so attn[t,t] = 1 / sum_s exp(s_ts - s_tt).  Rows where
some score exceeds the diagonal by >88 overflow exp to +inf; 1/inf = 0 matches
the true attention weight (~e^-100) to fp32 precision.  This removes every
reduce_max pass over the causal triangle (the DVE bottleneck of the previous
version).

Precision: all matmul inputs are fp16 (11-bit mantissa, values are O(1) so no
range issues) which runs at 1 cyc/row on the PE with no minimum moving width;
PSUM accumulation stays fp32.  Measured end-to-end rel err vs the fp32
reference: ~3.4e-3 (numpy emulation), well under the 2e-2 gate.

Per-core layout (core c: b = c//4, heads 2*(c%4), 2*(c%4)+1):
  - h[b] rows DMA'd to SBUF f32, cast to fp16, transposed via the XBAR DMA
    transpose (2-byte dtypes) into hT[d, t] -- no PE transposes, no PSUM
    round trip.
  - stage 1: gT[e, t] = sum_d A[d, e] hT[d, t], PSUM [128, 2048] tiles (one
    full e-chunk), single wide ACT copy PSUM->SBUF fp16.
  - stage 2 per 128-row tile: scores into one [128, 2048] PSUM tile (causal
    prefix 0..(i+1)*128); causal mask applied BY THE PE via one extra
    accumulated matmul identity16 x mask16 (-60000 above the diagonal);
    -diag extracted with one DVE mul + reduce; ONE wide ACT exp with
    bias=-diag and accum_out giving the row sum directly; DVE reciprocal and
    scale of the resident f32 h rows; output batched 4 tiles per DMA store.

Hardware notes (empirical, from the previous version of this kernel):
  - tensor_tensor_reduce with a PSUM input crashes the device; an ACT read of
    a PSUM region modified in place by the DVE also crashes.  PSUM here is
    written only by the PE; DVE/ACT only read it.
  - mask/identity constants are DMA'd from host inputs.
"""

import sys

try:
    import concourse.bass  # noqa: F401
except ImportError:  # pragma: no cover
    sys.path.insert(0, "/opt/trn_rl_repo")

import numpy as np

import concourse.bass as bass  # noqa: F401
import concourse.tile as tile
from concourse import bacc, bass_utils, mybir

B, T, D, H = 2, 2048, 512, 8
NCORES = 8
P = 128
NT = T // P      # 16 row tiles
ND = D // P      # 4 contraction chunks
SCH = 512        # PSUM bank width in fp32
MASKVAL = -60000.0  # fp16-representable; kills exp() after fp32 accumulation

f32 = mybir.dt.float32
f16 = mybir.dt.float16


def build_nc():
    nc = bacc.Bacc("TRN2", target_bir_lowering=False, debug=False)
    hb = nc.dram_tensor("hb", [T, D], f32, kind="ExternalInput")
    A2 = nc.dram_tensor("A2", [2, D, D], f32, kind="ExternalInput")
    mask16d = nc.dram_tensor("mask16d", [P, P], f16, kind="ExternalInput")
    ident16d = nc.dram_tensor("ident16d", [P, P], f16, kind="ExternalInput")
    negidentd = nc.dram_tensor("negidentd", [P, P], f32, kind="ExternalInput")
    out2 = nc.dram_tensor("out2", [2, T, D], f32, kind="ExternalOutput")
    hb_t = hb[:].rearrange("(n p) d -> p n d", p=P)      # [128, 16, 512] view
    out_t = out2[:].rearrange("h (n p) d -> p h n d", p=P)  # [128, 2, 16, 512]

    with tile.TileContext(nc) as tc:
        with (
            tc.tile_pool(name="const", bufs=1) as constp,
            tc.tile_pool(name="big", bufs=1) as big,
            tc.tile_pool(name="gpool", bufs=2) as gpool,
            tc.tile_pool(name="psum", bufs=2, space="PSUM") as psum,
            tc.tile_pool(name="expscr", bufs=2) as expscr,
            tc.tile_pool(name="stats", bufs=6) as stats,
            tc.tile_pool(name="outp", bufs=2) as outp,
        ):
            mask16 = constp.tile([P, P], f16)
            nc.gpsimd.dma_start(out=mask16, in_=mask16d[:])
            ident16 = constp.tile([P, P], f16)
            nc.gpsimd.dma_start(out=ident16, in_=ident16d[:])
            negident = constp.tile([P, P], f32)
            nc.gpsimd.dma_start(out=negident, in_=negidentd[:])

            # A: A_sb[p, hd, c, e] = A[hd, c*128 + p, e]; cast to fp16
            A_sb = big.tile([P, 2, ND, D], f32)
            nc.gpsimd.dma_start(
                out=A_sb, in_=A2[:].rearrange("h (c p) e -> p h c e", p=P)
            )
            A16 = big.tile([P, 2, ND, D], f16)
            for hd in range(2):
                nc.vector.tensor_copy(A16[:, hd], A_sb[:, hd])

            # h: load f32 (kept resident for the output scaling), cast fp16,
            # XBAR-transpose to hT16[p, c, t] = h[t, c*128 + p]
            h32 = big.tile([P, NT, D], f32)
            h16 = big.tile([P, NT, D], f16)
            hT16 = big.tile([P, ND, T], f16)
            for i in range(NT):
                nc.sync.dma_start(out=h32[:, i, :], in_=hb_t[:, i, :])
                nc.vector.tensor_copy(h16[:, i, :], h32[:, i, :])
                nc.sync.dma_start_transpose(
                    hT16[:, :, i * P : (i + 1) * P], h16[:, i, :]
                )

            for hd in range(2):
                # ---- stage 1: gT[e, t] = sum_d A[d, e] * hT[d, t] ----
                gT = gpool.tile([P, ND, T], f16, tag="gT")
                for ec in range(ND):
                    ecs = slice(ec * P, (ec + 1) * P)
                    pg = psum.tile([P, T], f32, tag="ps")
                    for dc in range(ND):
                        for tsl in range(T // SCH):
                            nc.tensor.matmul(
                                pg[:, tsl * SCH : (tsl + 1) * SCH],
                                A16[:, hd, dc, ecs],
                                hT16[:, dc, tsl * SCH : (tsl + 1) * SCH],
                                start=(dc == 0),
                                stop=(dc == ND - 1),
                            )
                    nc.scalar.copy(gT[:, ec, :], pg)

                # ---- stage 2 + diag softmax, per row tile ----
                for i in range(NT):
                    its = slice(i * P, (i + 1) * P)
                    wc = (i + 1) * P          # causal width of this row tile
                    nfull = i // 4            # full 512-wide chunks
                    base = nfull * SCH
                    wlast = wc - base         # 128..512
                    ps = psum.tile([P, T], f32, tag="ps")
                    for ec in range(ND):
                        for j in range(nfull):
                            nc.tensor.matmul(
                                ps[:, j * SCH : (j + 1) * SCH],
                                gT[:, ec, its],
                                hT16[:, ec, j * SCH : (j + 1) * SCH],
                                start=(ec == 0),
                                stop=(ec == ND - 1),
                            )
                        nc.tensor.matmul(
                            ps[:, base : base + wlast],
                            gT[:, ec, its],
                            hT16[:, ec, base : base + wlast],
                            start=(ec == 0),
                            stop=False,
                        )
                    # causal mask by the PE: ps[t, s] += mask16[t, s] on the
                    # diagonal block (-60000 strictly above the diagonal)
                    nc.tensor.matmul(
                        ps[:, i * P : (i + 1) * P],
                        ident16,
                        mask16,
                        start=False,
                        stop=True,
                    )
                    # -diag: mul diagonal block by -I, row-reduce
                    dscr = stats.tile([P, P], f32, tag="dscr")
                    nc.vector.tensor_mul(dscr, ps[:, i * P : (i + 1) * P], negident)
                    negdiag = stats.tile([P, 1], f32, tag="negdiag")
                    nc.vector.reduce_sum(
                        out=negdiag, in_=dscr, axis=mybir.AxisListType.X
                    )
                    # one wide exp over the causal prefix; row sums via accum
                    ex = expscr.tile([P, T], f32, tag="ex")
                    lsum = stats.tile([P, 1], f32, tag="lsum")
                    nc.scalar.activation(
                        out=ex[:, :wc],
                        in_=ps[:, :wc],
                        func=mybir.ActivationFunctionType.Exp,
                        bias=negdiag,
                        scale=1.0,
                        accum_out=lsum,
                    )
                    rl = stats.tile([P, 1], f32, tag="rl")
                    nc.vector.reciprocal(rl, lsum)
                    # out rows = attn_diag * h rows; batch 4 tiles per store
                    if i % 4 == 0:
                        ob = outp.tile([P, 4, D], f32, tag="ob")
                    nc.vector.tensor_scalar_mul(ob[:, i % 4, :], h32[:, i, :], rl)
                    if i % 4 == 3:
                        q = i // 4
                        nc.sync.dma_start(
                            out=out_t[:, hd, 4 * q : 4 * q + 4, :], in_=ob
                        )

    nc.compile()
    return nc


_NC_CACHE = {}


def _get_nc():
    if "nc" not in _NC_CACHE:
        _NC_CACHE["nc"] = build_nc()
    return _NC_CACHE["nc"]


def _consts():
    mask16 = np.triu(np.full((P, P), MASKVAL, np.float16), 1)
    ident16 = np.eye(P, dtype=np.float16)
    negident = -np.eye(P, dtype=np.float32)
    return mask16, ident16, negident


def make_in_maps(h, A):
    h = np.ascontiguousarray(h, dtype=np.float32)
    A = np.ascontiguousarray(A, dtype=np.float32)
    mask16, ident16, negident = _consts()
    in_maps = []
    for c in range(NCORES):
        b = c // 4
        h0 = 2 * (c % 4)
        in_maps.append({
            "hb": h[b],
            "A2": np.ascontiguousarray(A[h0 : h0 + 2]),
            "mask16d": mask16,
            "ident16d": ident16,
            "negidentd": negident,
        })
    return in_maps


def assemble(results):
    full = np.empty((B, H, T, D), dtype=np.float32)
    for c in range(NCORES):
        b = c // 4
        h0 = 2 * (c % 4)
        o = results[c]["out2"]
        full[b, h0] = o[0]
        full[b, h0 + 1] = o[1]
    return full.reshape(B, T, H * D)


def kernel(h, A):
    nc = _get_nc()
    res = bass_utils.run_bass_kernel_spmd(
        nc, make_in_maps(h, A), core_ids=list(range(NCORES))
    )
    return assemble(res.results)
